# revision 1
# baseline (speedup 1.0000x reference)
"""Causal self-attention kernel for 8 trn2 NeuronCores (Bass/Tile), v3.

Problem: B=4, T=2048, C=1024, H=16 heads, D=64. f32 in/out.
  qkv = x @ w_attn.T + b_attn ; causal softmax attention ; y @ w_proj.T + b_proj

Sharding: core i handles batch b=i//2, head-group g=i%2 (8 heads each).
Each core computes a partial projection output [T, C]; the host sums the
two head-group partials per batch and adds b_proj (exact in fp32).

v3 = v2 (all-bf16 pipeline) + a FLAT chunk pipeline across reps: K^T/V_aug
quarters are parity double-buffered, so the QKV projection of chunk gc+1
(even across a rep boundary) and the output projection of chunk gc-1
interleave into attention(gc)'s k-loop as PE fillers with no rep-boundary
bubble. exp streams on ACT; mask = one merged-head 0/1 multiply on Pool per
diagonal block; normalization copies O to SBUF first (frees the PSUM pair),
then reciprocal -> one DRAM bounce -> one partition-broadcast DMA -> bf16
multiplies on DVE.
"""
import sys
sys.path.insert(0, "/opt/trn_rl_repo")

from contextlib import ExitStack

import numpy as np

import concourse.bass as bass
import concourse.tile as tile
from concourse import bacc, mybir
from concourse.bass_utils import run_bass_kernel_spmd

F32 = mybir.dt.float32
BF16 = mybir.dt.bfloat16
EXP = mybir.ActivationFunctionType.Exp
N_CORES = 8
B, T, C = 4, 2048, 1024
H, D = 16, 64          # global heads
HL = 8                 # heads per core
CL = HL * D            # 512 local channels


def build_nc(reps: int = 1):
    """Build the SPMD Bass program (same on all cores)."""
    nc = bacc.Bacc("TRN2", target_bir_lowering=False, debug=False,
                   num_devices=N_CORES)
    xT_d = nc.dram_tensor("xT", [C, T], BF16, kind="ExternalInput").ap()
    wqkT_d = nc.dram_tensor("wqkT", [C, 2 * CL], BF16, kind="ExternalInput").ap()
    wvT_d = nc.dram_tensor("wvT", [C, CL], BF16, kind="ExternalInput").ap()
    bqk_d = nc.dram_tensor("bqk", [2 * CL, 1], F32, kind="ExternalInput").ap()
    bv_d = nc.dram_tensor("bv", [128, CL], BF16, kind="ExternalInput").ap()
    wpT_d = nc.dram_tensor("wpT", [CL, C], BF16, kind="ExternalInput").ap()
    tri_d = nc.dram_tensor("tri", [128, 128], BF16, kind="ExternalInput").ap()
    one_d = nc.dram_tensor("one", [128, 128], BF16, kind="ExternalInput").ap()
    out_d = nc.dram_tensor("out", [T, C], F32, kind="ExternalOutput").ap()

    xT_r = xT_d.rearrange("(c p) t -> p c t", p=128)
    total = reps * 4

    with tile.TileContext(nc) as tc, ExitStack() as top:
        top.enter_context(nc.allow_low_precision(
            reason="bf16 pipeline validated vs f32 reference: rel err ~3e-3"))
        persist = top.enter_context(tc.tile_pool(name="persist", bufs=1))
        # parity-double-buffered K^T / V_aug quarters
        # K^T: [part (2 heads of pair m), m, s-cols]
        kt_q = [[persist.tile([128, 4, 512], BF16, name=f"ktq{p}{q}",
                              tag=f"ktq{p}{q}") for q in range(4)]
                for p in range(2)]
        # V_aug: [s-part, s-block, 8 heads x (64 v | ones)]
        v_q = [[persist.tile([128, 4, 8 * 65], BF16, name=f"vq{p}{q}",
                             tag=f"vq{p}{q}") for q in range(4)]
               for p in range(2)]
        w_sb = persist.tile([128, 8, 3 * CL], BF16)   # qk | v weights
        wp_sb = persist.tile([128, 4, C], BF16)
        bqk_sb = persist.tile([128, 8], F32)
        bv_sb = persist.tile([128, CL], BF16)
        tri_sb = persist.tile([128, 128], BF16)

        for ot in range(8):
            nc.sync.dma_start(out=bqk_sb[:, ot:ot + 1],
                              in_=bqk_d[ot * 128:(ot + 1) * 128, :])
        nc.sync.dma_start(out=bv_sb[:], in_=bv_d[:])
        nc.sync.dma_start(out=tri_sb[:], in_=tri_d[:])
        for p in range(2):
            for q in range(4):
                vdst = v_q[p][q][:, :, :].rearrange(
                    "p k (h x) -> p k h x", x=65)[:, :, :, 64:65]
                nc.sync.dma_start(
                    out=vdst,
                    in_=one_d[:, q * 32:(q + 1) * 32].rearrange(
                        "p (k h) -> p k h", h=8)[:, :, :, None])
        nc.gpsimd.dma_start(
            out=w_sb[:, :, 0:2 * CL],
            in_=wqkT_d.rearrange("(c p) o -> p c o", p=128))
        nc.gpsimd.dma_start(
            out=w_sb[:, :, 2 * CL:3 * CL],
            in_=wvT_d.rearrange("(c p) o -> p c o", p=128))
        nc.gpsimd.dma_start(
            out=wp_sb[:],
            in_=wpT_d.rearrange("(c p) o -> p c o", p=128))

        with tc.tile_pool(name="xtp", bufs=2) as xtp, \
             tc.tile_pool(name="qtp", bufs=2) as qtp, \
             tc.tile_pool(name="ytp", bufs=2) as ytp, \
             tc.tile_pool(name="ptp", bufs=2) as ptp, \
             tc.tile_pool(name="osb", bufs=2) as osb, \
             tc.tile_pool(name="oc2", bufs=2) as oc2, \
             tc.tile_pool(name="ph2", bufs=2) as ph2, \
             tc.tile_pool(name="ps1", bufs=2, space="PSUM") as ps1, \
             tc.tile_pool(name="stp", bufs=2, space="PSUM") as stp, \
             tc.tile_pool(name="otp", bufs=1, space="PSUM") as otp, \
             tc.tile_pool(name="drp", bufs=2, space="DRAM") as drp:

            stores = {}
            yts = {}

            def load_thunk(gc):
                def load():
                    xt = xtp.tile([128, 8, 512], BF16, tag="xt")
                    stores[gc] = {"xt": xt}
                    t0 = (gc % 4) * 512
                    nc.sync.dma_start(out=xt[:], in_=xT_r[:, :, t0:t0 + 512])
                return load

            def qkv_thunks(gc):
                par = (gc // 4) % 2
                q = gc % 4
                ths = []

                def prep():
                    stores[gc]["qt"] = qtp.tile([128, 4, 512], BF16,
                                                name="qt", tag="qt")
                ths.append(prep)
                for ot in range(8):
                    def g(ot=ot):
                        xt = stores[gc]["xt"]
                        ps = ps1.tile([128, 512], F32, tag="ps1")
                        for c in range(8):
                            nc.tensor.matmul(
                                ps[:],
                                w_sb[:, c, ot * 128:(ot + 1) * 128],
                                xt[:, c, :],
                                start=(c == 0), stop=(c == 7))
                        if ot < 4:
                            dst = stores[gc]["qt"][:, ot, :]
                        else:
                            dst = kt_q[par][q][:, ot - 4, :]
                        nc.vector.tensor_scalar_add(dst, ps[:],
                                                    bqk_sb[:, ot:ot + 1])
                    ths.append(g)
                for vt in range(4):
                    def gv(vt=vt):
                        xt = stores[gc]["xt"]
                        ps = ps1.tile([128, 512], F32, tag="ps1")
                        for c in range(8):
                            nc.tensor.matmul(
                                ps[:],
                                xt[:, c, vt * 128:(vt + 1) * 128],
                                w_sb[:, c, 2 * CL:3 * CL],
                                start=(c == 0), stop=(c == 7))
                        vk = v_q[par][q][:, vt, :].rearrange(
                            "p (h x) -> p h x", x=65)[:, :, 0:64]
                        nc.vector.tensor_add(
                            vk, ps[:].rearrange("p (h x) -> p h x", x=64),
                            bv_sb[:].rearrange("p (h x) -> p h x", x=64))
                    ths.append(gv)
                return ths

            def proj_thunks(gc):
                tq = gc % 4
                yt = yts[gc]
                ths = []
                for tb4 in range(4):
                    def g(tb4=tb4, yt=yt):
                        ob = osb.tile([128, 1024], F32, tag="o")
                        for o2 in range(2):
                            ps = ps1.tile([128, 512], F32, tag="ps1")
                            for hc in range(4):
                                nc.tensor.matmul(
                                    ps[:],
                                    yt[:, hc, tb4 * 128:(tb4 + 1) * 128],
                                    wp_sb[:, hc, o2 * 512:(o2 + 1) * 512],
                                    start=(hc == 0), stop=(hc == 3))
                            nc.vector.tensor_copy(
                                ob[:, o2 * 512:(o2 + 1) * 512], ps[:])
                        nc.sync.dma_start(
                            out=out_d[tq * 512 + tb4 * 128:
                                      tq * 512 + (tb4 + 1) * 128, :],
                            in_=ob[:])
                    ths.append(g)
                return ths

            load_thunk(0)()
            load_thunk(1)()
            for th in qkv_thunks(0):
                th()
            for gc in range(total):
                tq = gc % 4
                par = (gc // 4) % 2
                tcs = tq * 512
                kmaxc = (tcs + 512) // 128
                qt = stores[gc]["qt"]
                yt_c = ytp.tile([128, 4, 512], BF16, tag="yt")
                yts[gc] = yt_c
                fillers = []
                if gc + 2 < total:
                    fillers.append(load_thunk(gc + 2))
                a = qkv_thunks(gc + 1) if gc + 1 < total else []
                b = proj_thunks(gc - 1) if gc > 0 else []
                for i in range(max(len(a), len(b))):
                    if i < len(a):
                        fillers.append(a[i])
                    if i < len(b):
                        fillers.append(b[i])
                slots = 4 * kmaxc
                emitted = 0
                slot = 0
                for m in range(4):
                    # head pair A=2m (partitions 0:64), B=2m+1 (64:128)
                    ot_A = otp.tile([65, 512], F32, tag="otA")
                    ot_B = otp.tile([65, 512], F32, tag="otB")

                    def emit_ot(k, pt):
                        # software-pipelined: consumes pt of iteration k
                        t_lo = 128 * k
                        lo = max(tcs, t_lo)
                        kq, kk = k // 4, k % 4
                        for g, ot_g in ((0, ot_A), (1, ot_B)):
                            nc.tensor.matmul(
                                ot_g[0:65, lo - tcs:512],
                                v_q[par][kq][:, kk, (2 * m + g) * 65:
                                             (2 * m + g) * 65 + 65],
                                pt[:, g * 512 + lo - tcs:g * 512 + 512],
                                start=(k == 0), stop=(k == kmaxc - 1))

                    prev = None
                    for k in range(kmaxc):
                        t_lo = 128 * k
                        lo = max(tcs, t_lo)
                        kq, kk = k // 4, k % 4
                        st = stp.tile([128, 1024], F32, tag="st")
                        for g, r0 in ((0, 0), (1, 64)):
                            nc.tensor.matmul(
                                st[:, g * 512 + lo - tcs:g * 512 + 512],
                                kt_q[par][kq][r0:r0 + 64, m,
                                              kk * 128:(kk + 1) * 128],
                                qt[r0:r0 + 64, m, lo - tcs:512],
                                start=True, stop=True)
                        pt = ptp.tile([128, 1024], BF16, tag="pt")
                        st3 = st[:].rearrange("p (g x) -> p g x", g=2)
                        pt3 = pt[:].rearrange("p (g x) -> p g x", g=2)
                        nc.scalar.activation(
                            pt3[:, :, lo - tcs:512],
                            st3[:, :, lo - tcs:512],
                            EXP, scale=0.125)
                        if tcs <= t_lo:
                            # zero the above-diagonal triangle of the
                            # diagonal block, both heads in one op
                            dc = t_lo - tcs
                            ptv = pt3[:, :, dc:dc + 128]
                            triv = bass.AP(
                                tensor=tri_sb.tensor,
                                offset=tri_sb[:].offset,
                                ap=[list(tri_sb[:].ap[0]), [0, 2],
                                    list(tri_sb[:].ap[1])])
                            nc.gpsimd.tensor_mul(ptv, ptv, triv)
                        if prev is not None:
                            emit_ot(prev[0], prev[1])
                        prev = (k, pt)
                        slot += 1
                        want = len(fillers) * slot // slots
                        while emitted < want:
                            fillers[emitted]()
                            emitted += 1
                    emit_ot(prev[0], prev[1])
                    # copy O to SBUF (frees the PSUM pair), then
                    # normalize: yt = o[0:64] / denom
                    oc = oc2.tile([64, 2, 512], BF16, tag="oc")
                    den = ph2.tile([1, 1024], BF16, tag="den")
                    nc.vector.reciprocal(den[:, 0:512], ot_A[64:65, :])
                    nc.vector.reciprocal(den[:, 512:1024], ot_B[64:65, :])
                    nc.vector.tensor_copy(oc[:, 0, :], ot_A[0:64, :])
                    nc.vector.tensor_copy(oc[:, 1, :], ot_B[0:64, :])
                    dbo = drp.tile([1, 1024], BF16, tag="dbo")
                    nc.sync.dma_start(out=dbo[:], in_=den[:])
                    rep_t = ph2.tile([64, 1024], BF16, tag="rep")
                    dap = dbo[0:1, :]
                    bc = bass.AP(tensor=dap.tensor, offset=dap.offset,
                                 ap=[[0, 64], [1, 1024]])
                    nc.gpsimd.dma_start(out=rep_t[:], in_=bc)
                    for g in (0, 1):
                        nc.vector.tensor_mul(
                            yt_c[g * 64:g * 64 + 64, m, :],
                            oc[:, g, :], rep_t[:, g * 512:g * 512 + 512])
                while emitted < len(fillers):
                    fillers[emitted]()
                    emitted += 1
                if gc - 2 >= 0:
                    del yts[gc - 2]
                    del stores[gc - 1]
            for th in proj_thunks(total - 1):
                th()
    nc.compile()
    return nc


def make_in_maps(x, w_attn, b_attn, w_proj):
    bf = mybir.dt.np(BF16)
    # pt layout is [s, q]: keep s <= q -> upper triangle incl. diagonal
    tri = np.triu(np.ones((128, 128), dtype=np.float32)).astype(bf)
    in_maps = []
    xTs = [np.ascontiguousarray(x[b].T).astype(bf) for b in range(B)]
    ones = np.ones((128, 128), dtype=np.float32).astype(bf)
    for i in range(N_CORES):
        b, g = i // 2, i % 2
        sl = slice(CL * g, CL * g + CL)
        wq = w_attn[0 * C:1 * C][sl.start:sl.stop]
        wk = w_attn[1 * C:2 * C][sl.start:sl.stop]
        wv = w_attn[2 * C:3 * C][sl.start:sl.stop]
        in_maps.append({
            "xT": xTs[b],
            "wqkT": np.ascontiguousarray(
                np.concatenate([wq, wk], 0).T).astype(bf),
            "wvT": np.ascontiguousarray(wv.T).astype(bf),
            "bqk": np.concatenate(
                [b_attn[0 * C:1 * C][sl.start:sl.stop],
                 b_attn[1 * C:2 * C][sl.start:sl.stop]]).reshape(2 * CL, 1)
                .astype(np.float32),
            "bv": np.broadcast_to(b_attn[2 * C:3 * C][sl.start:sl.stop],
                                  (128, CL)).astype(bf).copy(),
            "wpT": np.ascontiguousarray(w_proj[:, sl.start:sl.stop].T)
                .astype(bf),
            "tri": tri,
            "one": ones,
        })
    return in_maps


_NC_CACHE = {}


def kernel(x, w_attn, b_attn, w_proj, b_proj):
    x = np.asarray(x, dtype=np.float32)
    w_attn = np.asarray(w_attn, dtype=np.float32)
    b_attn = np.asarray(b_attn, dtype=np.float32)
    w_proj = np.asarray(w_proj, dtype=np.float32)
    b_proj = np.asarray(b_proj, dtype=np.float32)

    if "nc" not in _NC_CACHE:
        _NC_CACHE["nc"] = build_nc()
    nc = _NC_CACHE["nc"]
    in_maps = make_in_maps(x, w_attn, b_attn, w_proj)
    res = run_bass_kernel_spmd(nc, in_maps, list(range(N_CORES))).results
    out = np.empty((B, T, C), dtype=np.float32)
    for b in range(B):
        out[b] = res[2 * b]["out"] + res[2 * b + 1]["out"] + b_proj
    return out

